# revision 9
# baseline (speedup 1.0000x reference)
"""CausalWanSelfAttention Trainium2 kernel — 8-core SPMD, 3 phases.

Phase A (column-sharded, 192 cols/core of each of q/k/v): QKV projections in
  bf16 (each core loads only its weight slice), bias, RoPE (pre-norm; rotation
  commutes with the per-token rms scale), partial sum-of-squares for the rms
  norm (full-row sums need a cross-core reduce, done in phase B on stacked
  partials).
Phase B (unit-sharded attention): 48 units of (head, 390-query block), 6 per
  core grouped into 3 slots of (head, 780-half). Scores in f32r with keys on
  psum partitions; exp (f32->fp16) over 3 key-blocks at a time to amortize the
  ACT fixed cost; softmax denominator accumulated on DVE in fp16 and reduced
  across partitions on GPSIMD (no psum, no tensor-engine ones-matmul); AV in
  fp16 pipelined 2 triples behind the scores.
Phase C (column-sharded, 192 cols/core): output projection in bf16 + bias.

Host code between phases only reshapes / transposes / casts / concatenates.
"""

import numpy as np

import concourse.bass as bass
import concourse.bacc as bacc
import concourse.bass_isa as bass_isa
import concourse.mybir as mybir
import concourse.tile as tile
from concourse.bass_utils import run_bass_kernel_spmd

F32 = mybir.dt.float32
F32R = mybir.dt.float32r
BF16 = mybir.dt.bfloat16
FP16 = mybir.dt.float16

N_CORES = 8
DIM = 1536
NH = 12
HD = 128
S = 1560
CUR_START = 4680
WIN = CUR_START + S          # 6240 attended keys
N_KB = (WIN + 127) // 128    # 49 key blocks
LAST_KP = WIN - (N_KB - 1) * 128   # 96
N_TRI = 16                   # full triples of key blocks (last kb solo)
QW = 390                     # query width per attention unit
SLOTS = 3                    # (head, 780-half) slots per core
UNITS = 6                    # (head, 390-quarter) units per core
CPC = DIM // N_CORES         # 192 cols per core in phases A and C
RG = 120                     # row-group tokens (13 groups of 120)
N_RG = S // RG
EPS = 1e-6
SCALE = float(1.0 / np.sqrt(HD))

_programs = {}


def _bcast_rows(handle, n, rows=128):
    """AP reading a [n] DRAM tensor broadcast across `rows` partitions."""
    return bass.AP(tensor=handle, offset=0, ap=[[0, rows], [1, n]])


def _swap_pairs(ap_2d, rows, pairs):
    """View of [rows, pairs*2] with each (even,odd) pair swapped."""
    p_step = ap_2d.ap[0][0]
    return bass.AP(
        tensor=ap_2d.tensor,
        offset=ap_2d.offset + 1,
        ap=[[p_step, rows], [2, pairs], [-1, 2]],
    )


# --------------------------------------------------------------------------
# Phase A: x -> raw q/k/v column slices (roped, un-normed) + sumsq partials
# --------------------------------------------------------------------------
def _build_A():
    nc = bacc.Bacc()
    xT = nc.dram_tensor("xT", [NH, 128, S], BF16, kind="ExternalInput")
    wT = nc.dram_tensor("wT", [3, NH, 128, CPC], BF16, kind="ExternalInput")
    b_in = nc.dram_tensor("b3", [3, CPC], F32, kind="ExternalInput")
    g_in = nc.dram_tensor("g2", [2, CPC], F32, kind="ExternalInput")
    ang_in = nc.dram_tensor("ang", [S, CPC // 2], F32, kind="ExternalInput")
    outs = {w: nc.dram_tensor(f"{w}_out", [S, CPC], F32, kind="ExternalOutput")
            for w in ("q", "k", "v")}
    ssq_out = nc.dram_tensor("ssq", [2, S], F32, kind="ExternalOutput")

    NP = CPC // 2  # 96 rope pairs per core

    with tile.TileContext(nc) as tc:
        with (
            tc.tile_pool(name="consts", bufs=1) as consts,
            tc.tile_pool(name="rope", bufs=2) as rope,
            tc.tile_pool(name="acts", bufs=2) as acts,
            tc.tile_pool(name="small", bufs=4) as small,
            tc.tile_pool(name="psum", bufs=3, space="PSUM") as psum,
        ):
            eps_t = consts.tile([128, 1], F32, tag="epsc")
            nc.vector.memset(eps_t, EPS)
            xt = consts.tile([128, NH, S], BF16, tag="xT")
            nc.sync.dma_start(out=xt, in_=xT.ap().rearrange("k p r -> p k r"))
            wt = consts.tile([128, 3, NH, CPC], BF16, tag="wT")
            nc.sync.dma_start(out=wt, in_=wT.ap().rearrange("w k p c -> p w k c"))
            bias_t = consts.tile([128, 3, CPC], F32, tag="bias")
            nc.gpsimd.dma_start(out=bias_t, in_=_bcast_rows(b_in, 3 * CPC))
            g_t = consts.tile([128, 2, CPC], F32, tag="g")
            nc.gpsimd.dma_start(out=g_t, in_=_bcast_rows(g_in, 2 * CPC))

            for rg in range(N_RG):
                r0 = rg * RG
                # ---- rope tables for this row block (96 pairs) ----
                ang_t = rope.tile([RG, NP], F32, tag="ang")
                nc.gpsimd.dma_start(out=ang_t, in_=ang_in[r0:r0 + RG])
                thc = rope.tile([RG, NP], F32, tag="thc")
                nc.vector.tensor_scalar_add(thc, ang_t, float(np.pi / 2))
                mc = rope.tile([RG, NP], F32, tag="mc")
                nc.vector.tensor_scalar(out=mc, in0=thc,
                                        scalar1=float(np.pi),
                                        scalar2=float(2 * np.pi),
                                        op0=mybir.AluOpType.is_ge,
                                        op1=mybir.AluOpType.mult)
                nc.vector.tensor_sub(thc, thc, mc)
                ths = rope.tile([RG, NP], F32, tag="ths")
                ms = rope.tile([RG, NP], F32, tag="ms")
                nc.vector.tensor_scalar(out=ms, in0=ang_t,
                                        scalar1=float(np.pi),
                                        scalar2=float(2 * np.pi),
                                        op0=mybir.AluOpType.is_ge,
                                        op1=mybir.AluOpType.mult)
                nc.vector.tensor_sub(ths, ang_t, ms)
                csb = rope.tile([RG, NP], F32, tag="csb")
                ssb = rope.tile([RG, NP], F32, tag="ssb")
                nc.scalar.activation(csb, thc, mybir.ActivationFunctionType.Sin)
                nc.scalar.activation(ssb, ths, mybir.ActivationFunctionType.Sin)
                cos2 = rope.tile([RG, NP, 2], F32, tag="cos2")
                sinp = rope.tile([RG, NP, 2], F32, tag="sinp")
                nc.vector.tensor_copy(cos2[:, :, 0], csb)
                nc.vector.tensor_copy(cos2[:, :, 1], csb)
                nc.scalar.mul(sinp[:, :, 0], ssb, -1.0)
                nc.vector.tensor_copy(sinp[:, :, 1], ssb)
                cos2f = cos2.rearrange("p c t -> p (c t)")
                sinpf = sinp.rearrange("p c t -> p (c t)")

                for wi, w in enumerate(("q", "k", "v")):
                    ps = psum.tile([RG, CPC], F32, tag="ps")
                    for kt in range(NH):
                        nc.tensor.matmul(
                            ps,
                            xt[:, kt, r0:r0 + RG],
                            wt[:, wi, kt, :],
                            start=(kt == 0),
                            stop=(kt == NH - 1),
                        )
                    t = acts.tile([RG, CPC], F32, tag=f"t{wi}")
                    nc.vector.tensor_add(t, ps, bias_t[:RG, wi, :])
                    if w == "v":
                        nc.gpsimd.dma_start(out=outs["v"][r0:r0 + RG], in_=t)
                        continue
                    # partial sum of squares (pre-g), accumulated per token
                    scr = acts.tile([RG, CPC], F32, tag="scr")
                    ssq_t = small.tile([RG, 1], F32, tag=f"ssq{wi}")
                    nc.scalar.activation(scr, t,
                                         mybir.ActivationFunctionType.Square,
                                         accum_out=ssq_t)
                    nc.gpsimd.dma_start(out=ssq_out[wi, r0:r0 + RG],
                                        in_=ssq_t.rearrange("p o -> (p o)"))
                    # g, then rope (pre-norm; rstd applied in phase B)
                    tg = acts.tile([RG, CPC], F32, tag="tg")
                    nc.vector.tensor_mul(tg, t, g_t[:RG, wi, :])
                    t1 = acts.tile([RG, CPC], F32, tag="t1")
                    nc.vector.tensor_mul(t1, tg, cos2f)
                    t2 = acts.tile([RG, CPC], F32, tag="t2")
                    nc.gpsimd.tensor_mul(t2, _swap_pairs(tg, RG, NP), sinpf)
                    rot = acts.tile([RG, CPC], F32, tag="rot")
                    nc.vector.tensor_add(rot, t1, t2)
                    nc.gpsimd.dma_start(out=outs[w][r0:r0 + RG], in_=rot)
    nc.finalize()
    return nc


# --------------------------------------------------------------------------
# Phase B: attention.  Per core: 3 slots x (kT [128, WIN] f32, v fp16),
# 6 units of 390 queries.  ssq partials reduced on-device for the rms scale.
# --------------------------------------------------------------------------
def _build_B():
    nc = bacc.Bacc()
    kT_in = nc.dram_tensor("kT", [SLOTS, 128, WIN], F32R, kind="ExternalInput")
    v_in = nc.dram_tensor("v", [SLOTS, 128, N_KB, HD], FP16,
                          kind="ExternalInput")
    qT_in = nc.dram_tensor("qT", [UNITS, 128, QW], F32R, kind="ExternalInput")
    ssq_q_in = nc.dram_tensor("ssq_q", [N_CORES, UNITS, QW], F32,
                              kind="ExternalInput")
    ssq_k_in = nc.dram_tensor("ssq_k", [N_CORES, S], F32, kind="ExternalInput")
    ao_out = nc.dram_tensor("aoT", [UNITS, 128, QW], F32, kind="ExternalOutput")

    with tile.TileContext(nc) as tc:
        with (
            tc.tile_pool(name="big", bufs=1) as big,
            tc.tile_pool(name="norm", bufs=1) as norm,
            tc.tile_pool(name="ep", bufs=4) as ep,
            tc.tile_pool(name="den", bufs=1) as denp,
            tc.tile_pool(name="aop", bufs=2) as aop,
            tc.tile_pool(name="ps_s", bufs=2, space="PSUM") as ps_s,
            tc.tile_pool(name="ps_o", bufs=2, space="PSUM") as ps_o,
        ):
            eps_t = norm.tile([128, 1], F32, tag="epsc")
            nc.vector.memset(eps_t, EPS)

            kt = big.tile([128, SLOTS, WIN], F32R, tag="kT")
            nc.sync.dma_start(out=kt, in_=kT_in.ap().rearrange("s p w -> p s w"))
            vt = big.tile([128, SLOTS, N_KB, HD], FP16, tag="v")
            nc.sync.dma_start(out=vt, in_=v_in.ap().rearrange("s p b d -> p s b d"))
            qt = big.tile([128, UNITS, QW], F32R, tag="qT")
            nc.sync.dma_start(out=qt, in_=qT_in.ap().rearrange("u p q -> p u q"))

            # ---- rms scales from stacked partials (sqrt/recip in place) ----
            ssq_q = norm.tile([N_CORES, UNITS * QW], F32, tag="ssq_q")
            nc.gpsimd.dma_start(
                out=ssq_q, in_=ssq_q_in.ap().rearrange("c u q -> c (u q)"))
            ssq_k = norm.tile([N_CORES, S], F32, tag="ssq_k")
            nc.gpsimd.dma_start(out=ssq_k, in_=ssq_k_in.ap())
            red_q = norm.tile([N_CORES, UNITS * QW], F32, tag="red_q")
            nc.gpsimd.partition_all_reduce(red_q, ssq_q, channels=N_CORES,
                                           reduce_op=bass_isa.ReduceOp.add)
            red_k = norm.tile([N_CORES, S], F32, tag="red_k")
            nc.gpsimd.partition_all_reduce(red_k, ssq_k, channels=N_CORES,
                                           reduce_op=bass_isa.ReduceOp.add)
            nc.scalar.activation(red_q[:1], red_q[:1],
                                 mybir.ActivationFunctionType.Sqrt,
                                 scale=1.0 / DIM, bias=eps_t[:1])
            nc.scalar.activation(red_k[:1], red_k[:1],
                                 mybir.ActivationFunctionType.Sqrt,
                                 scale=1.0 / DIM, bias=eps_t[:1])
            nc.vector.reciprocal(red_q[:1], red_q[:1])
            nc.vector.reciprocal(red_k[:1], red_k[:1])
            rq_bc = norm.tile([128, UNITS, QW], F32, tag="rq_bc")
            nc.gpsimd.partition_broadcast(
                rq_bc.rearrange("p u q -> p (u q)"), red_q[:1])
            rk_bc = norm.tile([128, S], F32, tag="rk_bc")
            nc.gpsimd.partition_broadcast(rk_bc, red_k[:1])

            # normalize new k columns and all q
            for s in range(SLOTS):
                nc.vector.tensor_mul(kt[:, s, CUR_START:WIN],
                                     kt[:, s, CUR_START:WIN], rk_bc)
            for u in range(UNITS):
                nc.vector.tensor_mul(qt[:, u, :], qt[:, u, :], rq_bc[:, u, :])

            # ---- attention units, AV lagging 2 triples behind scores ----
            ntile = N_TRI + 1            # 16 triples + 1 solo (kb 48)
            es = {}                      # (u, t) -> exp tile
            po = {}                      # u -> psum accumulator
            den = {}                     # u -> fp16 denominator accum

            def emit_scores(u, t):
                s = u // 2
                sc = ps_s.tile([128, 3, 512], F32, tag="sc")
                nkb = 3 if t < N_TRI else 1
                kp = 128
                for j in range(nkb):
                    kb = 3 * t + j
                    kp = 128 if kb < N_KB - 1 else LAST_KP
                    nc.tensor.matmul(
                        sc[:kp, j, :QW],
                        kt[:, s, kb * 128: kb * 128 + kp],
                        qt[:, u, :],
                        start=True, stop=True, skip_group_check=True,
                    )
                e = ep.tile([128, 3, QW], FP16, tag="e")
                if t < N_TRI:
                    nc.scalar.activation(e, sc[:, :, :QW],
                                         mybir.ActivationFunctionType.Exp,
                                         scale=SCALE)
                    d = den[u]
                    nc.vector.tensor_add(d, d, e[:, 0, :])
                    nc.vector.tensor_add(d, d, e[:, 1, :])
                    nc.vector.tensor_add(d, d, e[:, 2, :])
                else:
                    nc.scalar.activation(e[:kp, 0, :], sc[:kp, 0, :QW],
                                         mybir.ActivationFunctionType.Exp,
                                         scale=SCALE)
                    nc.vector.tensor_add(den[u][:kp], den[u][:kp], e[:kp, 0, :])
                es[u, t] = e

            def emit_av(u, t):
                s = u // 2
                nkb = 3 if t < N_TRI else 1
                e = es.pop((u, t))
                for j in range(nkb):
                    kb = 3 * t + j
                    kp = 128 if kb < N_KB - 1 else LAST_KP
                    nc.tensor.matmul(
                        po[u][:, :QW],
                        vt[:kp, s, kb, :],
                        e[:kp, j, :],
                        start=(kb == 0), stop=(kb == N_KB - 1),
                        skip_group_check=True,
                    )

            def emit_finish(u):
                dr = aop.tile([128, QW], F32, tag="dr")
                nc.gpsimd.partition_all_reduce(dr, den[u], channels=128,
                                               reduce_op=bass_isa.ReduceOp.add)
                rden = aop.tile([128, QW], F32, tag="rden")
                nc.vector.reciprocal(rden, dr)
                ao = aop.tile([128, QW], F32, tag="ao")
                nc.vector.tensor_mul(ao, po[u][:, :QW], rden)
                nc.gpsimd.dma_start(out=ao_out[u], in_=ao)

            LAG = 2
            for u in range(UNITS):
                den[u] = denp.tile([128, QW], FP16, tag=f"den{u}",
                                   name=f"den{u}")
                nc.vector.memset(den[u], 0.0)
                po[u] = ps_o.tile([128, 512], F32, tag="po", name=f"po{u}")
                for t in range(ntile):
                    emit_scores(u, t)
                    if t >= LAG:
                        emit_av(u, t - LAG)
                for t in range(ntile - LAG, ntile):
                    emit_av(u, t)
                emit_finish(u)
                if u >= 1:
                    del po[u - 1], den[u - 1]
    nc.finalize()
    return nc


# --------------------------------------------------------------------------
# Phase C: out[:, cols] = attn_out @ wo.T[:, cols] + bo[cols]
# --------------------------------------------------------------------------
def _build_C():
    nc = bacc.Bacc()
    aT_in = nc.dram_tensor("aT", [NH, 128, S], BF16, kind="ExternalInput")
    woT_in = nc.dram_tensor("woT", [NH, 128, CPC], BF16, kind="ExternalInput")
    bo_in = nc.dram_tensor("bo", [CPC], F32, kind="ExternalInput")
    out = nc.dram_tensor("out", [S, CPC], F32, kind="ExternalOutput")

    with tile.TileContext(nc) as tc:
        with (
            tc.tile_pool(name="consts", bufs=1) as consts,
            tc.tile_pool(name="acts", bufs=3) as acts,
            tc.tile_pool(name="psum", bufs=4, space="PSUM") as psum,
        ):
            at = consts.tile([128, NH, S], BF16, tag="aT")
            nc.sync.dma_start(out=at, in_=aT_in.ap().rearrange("k p r -> p k r"))
            wo_t = consts.tile([128, NH, CPC], BF16, tag="woT")
            nc.sync.dma_start(out=wo_t,
                              in_=woT_in.ap().rearrange("k p c -> p k c"))
            bo_t = consts.tile([128, CPC], F32, tag="bo")
            nc.gpsimd.dma_start(out=bo_t, in_=_bcast_rows(bo_in, CPC))

            for rg in range(N_RG):
                r0 = rg * RG
                ps = psum.tile([RG, CPC], F32, tag="ps")
                for kt in range(NH):
                    nc.tensor.matmul(
                        ps,
                        at[:, kt, r0:r0 + RG],
                        wo_t[:, kt, :],
                        start=(kt == 0),
                        stop=(kt == NH - 1),
                    )
                osb = acts.tile([RG, CPC], F32, tag="osb")
                nc.vector.tensor_add(osb, ps, bo_t[:RG])
                nc.gpsimd.dma_start(out=out[r0:r0 + RG], in_=osb)
    nc.finalize()
    return nc


def _get_program(name):
    if name not in _programs:
        _programs[name] = {"A": _build_A, "B": _build_B, "C": _build_C}[name]()
    return _programs[name]


# --------------------------------------------------------------------------
# Host orchestration
# --------------------------------------------------------------------------
def _build_angles(freqs, grid, start_frame):
    """[S, 64] per-position rope angles — pure indexing of `freqs`."""
    F_, H_, W_ = grid
    c = HD // 2          # 64
    c3 = c // 3          # 21
    f_ang = freqs[start_frame:start_frame + F_, : c - 2 * c3]
    h_ang = freqs[:H_, c - 2 * c3: c - c3]
    w_ang = freqs[:W_, c - c3:]
    ang = np.concatenate([
        np.broadcast_to(f_ang[:, None, None, :], (F_, H_, W_, c - 2 * c3)),
        np.broadcast_to(h_ang[None, :, None, :], (F_, H_, W_, c3)),
        np.broadcast_to(w_ang[None, None, :, :], (F_, H_, W_, c3)),
    ], axis=-1).reshape(F_ * H_ * W_, c)
    return np.ascontiguousarray(ang, dtype=np.float32)


def _run(nc, in_maps, **kw):
    return run_bass_kernel_spmd(nc, in_maps, core_ids=list(range(N_CORES)), **kw)


def kernel(x, freqs, wq, bq, wk, bk, wv, bv, wo, bo, gq, gk,
           kv_cache_k, kv_cache_v, grid_sizes, seq_lens, current_start,
           _timings=None):
    x = np.asarray(x, dtype=np.float32)
    freqs = np.asarray(freqs, dtype=np.float32)
    grid = [int(v) for v in np.asarray(grid_sizes).reshape(-1)[:3]]
    cur = int(np.asarray(current_start))
    assert grid[0] * grid[1] * grid[2] == S and cur == CUR_START

    frame_seqlen = grid[1] * grid[2]
    start_frame = cur // frame_seqlen
    ang = _build_angles(freqs, grid, start_frame)   # [S, 64]

    trace = _timings is not None

    # ---------------- phase A ----------------
    xT_bf = _to_bf16(np.ascontiguousarray(x.reshape(S, DIM).T.reshape(NH, 128, S)))
    w_all = np.stack([np.asarray(w, np.float32).T for w in (wq, wk, wv)])
    b_all = np.stack([np.asarray(b, np.float32) for b in (bq, bk, bv)])
    g_all = np.stack([np.asarray(g, np.float32) for g in (gq, gk)])

    pA = _get_program("A")
    in_maps = []
    for c in range(N_CORES):
        cs = slice(c * CPC, (c + 1) * CPC)
        cols = np.arange(c * CPC, (c + 1) * CPC)
        pair_idx = (cols[::2] % HD) // 2          # 96 angle columns
        in_maps.append({
            "xT": xT_bf,
            "wT": _to_bf16(np.ascontiguousarray(
                w_all[:, :, cs].reshape(3, NH, 128, CPC))),
            "b3": np.ascontiguousarray(b_all[:, cs]),
            "g2": np.ascontiguousarray(g_all[:, cs]),
            "ang": np.ascontiguousarray(ang[:, pair_idx]),
        })
    rA = _run(pA, in_maps, trace=trace)

    q_rot = np.concatenate([rA.results[c]["q_out"] for c in range(N_CORES)], 1)
    k_rot = np.concatenate([rA.results[c]["k_out"] for c in range(N_CORES)], 1)
    v_new = np.concatenate([rA.results[c]["v_out"] for c in range(N_CORES)], 1)
    ssq_q = np.stack([rA.results[c]["ssq"][0] for c in range(N_CORES)])  # [8,S]
    ssq_k = np.stack([rA.results[c]["ssq"][1] for c in range(N_CORES)])

    # ---------------- host reshuffle for phase B ----------------
    cache_k = np.asarray(kv_cache_k, np.float32)[0, :cur]      # [4680, 12, 128]
    cache_v = np.asarray(kv_cache_v, np.float32)[0, :cur]
    k_rot_h = k_rot.reshape(S, NH, HD)
    v_new_h = v_new.reshape(S, NH, HD)

    kT_heads = np.empty((NH, HD, WIN), np.float32)
    kT_heads[:, :, :cur] = cache_k.transpose(1, 2, 0)
    kT_heads[:, :, cur:] = k_rot_h.transpose(1, 2, 0)
    v_tmp = np.zeros((NH, N_KB * 128, HD), np.float16)
    v_tmp[:, :WIN] = np.concatenate([cache_v, v_new_h]).transpose(1, 0, 2)
    # -> [head, key_in_block, block, d]
    v_pad = np.ascontiguousarray(
        v_tmp.reshape(NH, N_KB, 128, HD).transpose(0, 2, 1, 3))
    qT_heads = q_rot.reshape(S, NH, HD).transpose(1, 2, 0)     # [12, 128, S]

    pB = _get_program("B")
    in_maps = []
    for c in range(N_CORES):
        heads = [(3 * c + s) // 2 for s in range(SLOTS)]
        halves = [(3 * c + s) % 2 for s in range(SLOTS)]
        ssq_q_units = np.empty((N_CORES, UNITS, QW), np.float32)
        qTs = np.empty((UNITS, HD, QW), np.float32)
        for u in range(UNITS):
            s, j = u // 2, u % 2
            t0 = halves[s] * 780 + j * QW
            ssq_q_units[:, u, :] = ssq_q[:, t0:t0 + QW]
            qTs[u] = qT_heads[heads[s], :, t0:t0 + QW]
        in_maps.append({
            "kT": np.ascontiguousarray(kT_heads[heads]),
            "v": np.ascontiguousarray(v_pad[heads]),
            "qT": np.ascontiguousarray(qTs),
            "ssq_q": ssq_q_units,
            "ssq_k": ssq_k,
        })
    rB = _run(pB, in_maps, trace=trace)

    aoT = np.empty((NH, HD, S), np.float32)
    for c in range(N_CORES):
        for u in range(UNITS):
            s, j = u // 2, u % 2
            h = (3 * c + s) // 2
            t0 = ((3 * c + s) % 2) * 780 + j * QW
            aoT[h, :, t0:t0 + QW] = rB.results[c]["aoT"][u]

    # ---------------- phase C ----------------
    pC = _get_program("C")
    aT_bf = _to_bf16(np.ascontiguousarray(aoT))
    woT = np.asarray(wo, np.float32).T.reshape(NH, 128, DIM)
    bo = np.asarray(bo, np.float32)
    in_maps = []
    for c in range(N_CORES):
        cs = slice(c * CPC, (c + 1) * CPC)
        in_maps.append({
            "aT": aT_bf,
            "woT": _to_bf16(np.ascontiguousarray(woT[:, :, cs])),
            "bo": np.ascontiguousarray(bo[cs]),
        })
    rC = _run(pC, in_maps, trace=trace)
    out = np.concatenate([rC.results[c]["out"] for c in range(N_CORES)], 1)

    if _timings is not None:
        for name, r in (("A", rA), ("B", rB), ("C", rC)):
            _timings[name] = r.exec_time_ns
    return out.reshape(1, S, DIM)


def _to_bf16(a):
    import ml_dtypes
    return np.ascontiguousarray(a.astype(ml_dtypes.bfloat16))
